# revision 36
# baseline (speedup 1.0000x reference)
"""Multi-head self-attention forward on 8 Trainium2 NeuronCores.

Problem: x[4,2048,512] -> qkv proj (w_qkv [512,1536]) -> 8-head attention
(head_dim 64) -> out proj (w_out [512,512] + b_out) -> y[4,2048,512].

Sharding: 8 shards = (batch b in 0..3) x (head-group hg in 0..1, 4 heads each).
Core c handles b=c//2, hg=c%2. Host sums the two half-projections per batch
and adds the bias.

Kernel structure (per core):
  phase 1: qkT = wperm.T @ xT (4 tiles [128,2048]: Q01,K01,Q23,K23 with the
    pair's two heads stacked on partitions 0:64 / 64:128), v_aug seq-tiles
    [128, 4*65] (per head 64 v columns + ones column -> the AV matmul also
    emits the softmax denominator as output row 64).
  phase 2: per (pair p, q-chunk qc of 512): 16 kc iterations:
    - scores: two row-tiled (64x128 PE mode) matmuls, head A -> s[:,0:512],
      head B -> s[:,512:1024]; the 64-row tiles at tile_position (0,0)/(64,0)
      execute concurrently.
    - exp: split between ACT (activation Exp) and DVE (Schraudolph bit-trick:
      one tensor_scalar mul+add emitting int16 bf16-bit-patterns, bitcast to
      bf16) so neither engine is the bottleneck.
    - AV: 2 matmuls accumulate [V|1].T @ p into oA/oB [65, 512] psum.
    block end: rt = 1/den via DVE reciprocal from psum row 64, DMA broadcasts
    rt across 64 partitions, one fused DVE tensor_tensor per head scales the
    oA/oB psum by rt while evacuating to bf16 oT tiles (partitions 0:64).
  out-proj: per q-tile of 128: 4 accumulating K=64 matmuls (oT blocks x w2
    head-half rows), evac + DMA. Emitted after each pair-1 block (and tail).
"""

import math

import numpy as np

import concourse.bass as bass
import concourse.mybir as mybir
import concourse.tile as tile
from concourse import bacc

DIM = 512
NHEADS = 8
HD = 64
B = 4
SEQ = 2048
SCALE = HD ** -0.5

NCORES = 8
HPC = 4          # heads per core
QCH = 512        # q chunk
NQC = SEQ // QCH # 4 q-chunks
KCH = 128        # k chunk (psum partition dim)
NKC = SEQ // KCH # 16 k-chunks
CCH = 128        # contraction chunk for projections
NCC = DIM // CCH # 4

F32 = mybir.dt.float32
BF16 = mybir.dt.bfloat16
I16 = mybir.dt.int16
MMDT = BF16

# Schraudolph exp in bf16 bit-space: exp(s*SCALE) ~= bitcast_bf16(int16(
#   s * TRICK_A + TRICK_B)).  TRICK_A = 128*log2(e)*SCALE.  C_CORR centers
# the sawtooth approximation error (tuned vs rel-err on HW).
C_CORR = 7.33
TRICK_A = float(128.0 / math.log(2.0) * SCALE)
TRICK_B = float(127.0 * 128.0 - C_CORR)


def _dve_slot(p, qc, i):
    """Which exp units run on DVE (Schraudolph) vs ACT (exact).  The last
    two i keep DVE free for the block-end reciprocal/scale chain."""
    if i >= NKC - 2:
        return False
    return i % 8 in (1, 3, 4, 6) if (p, qc) != (0, 0) else i in (5, 9, 13)


def build_nc():
    nc = bacc.Bacc()

    xT_d = nc.dram_tensor("xt", [DIM, SEQ], MMDT, kind="ExternalInput")
    wperm_d = nc.dram_tensor("wperm", [DIM, 4 * 128], MMDT, kind="ExternalInput")
    wv_d = nc.dram_tensor("wv", [DIM, HPC * HD], MMDT, kind="ExternalInput")
    w2_d = nc.dram_tensor("w2", [HPC * HD, DIM], MMDT, kind="ExternalInput")
    y_d = nc.dram_tensor("y", [SEQ, DIM], F32, kind="ExternalOutput")

    with tile.TileContext(nc) as tc:
        with (
            tc.tile_pool(name="const", bufs=1) as cpool,
            tc.tile_pool(name="big", bufs=1) as bigpool,
            tc.tile_pool(name="pt", bufs=4) as ptpool,
            tc.tile_pool(name="small", bufs=2) as smallpool,
            tc.tile_pool(name="ysb", bufs=2) as ypool,
            tc.tile_pool(name="ps", bufs=1, space="PSUM") as ps,
        ):
            # ---- constants / inputs to SBUF ----
            xTs = [cpool.tile([128, SEQ], MMDT, tag=f"xT{c}", name=f"xT{c}")
                   for c in range(NCC)]
            wps = [cpool.tile([128, 512], MMDT, tag=f"wp{c}", name=f"wp{c}")
                   for c in range(NCC)]
            wvs = [cpool.tile([128, HPC * HD], MMDT, tag=f"wv{c}", name=f"wv{c}")
                   for c in range(NCC)]
            # w2 head-half rows, each on partitions 0:64
            w2h = [cpool.tile([64, DIM], MMDT, tag=f"w2h{t}", name=f"w2h{t}")
                   for t in range(4)]
            ones4 = cpool.tile([128, HPC * 64], F32, tag="ones4")
            nc.gpsimd.memset(ones4[:], 1.0)
            ones1 = cpool.tile([1, 1], F32, tag="ones1")
            nc.gpsimd.memset(ones1[:], 1.0)
            # preload exp ACT tables so the first attention exp doesn't stall
            dummy = cpool.tile([1, 1], F32, tag="dummy")
            nc.scalar.activation(dummy[:], ones1[:],
                                 mybir.ActivationFunctionType.Exp)

            nc.gpsimd.dma_start(wps[0][:, 0:128], wperm_d[0:128, 0:128])
            nc.gpsimd.dma_start(wps[0][:, 128:512], wperm_d[0:128, 128:512])
            nc.gpsimd.dma_start(xTs[0][:, 0:512], xT_d[0:128, 0:512])
            for c in range(NCC):
                nc.sync.dma_start(xTs[c][:, 0 if c else 512:1024],
                                  xT_d[c * 128:(c + 1) * 128, 0 if c else 512:1024])
            for c in range(1, NCC):
                nc.sync.dma_start(wps[c][:], wperm_d[c * 128:(c + 1) * 128, :])
            for c in range(NCC):
                nc.sync.dma_start(xTs[c][:, 1024:SEQ],
                                  xT_d[c * 128:(c + 1) * 128, 1024:SEQ])
                nc.sync.dma_start(wvs[c][:], wv_d[c * 128:(c + 1) * 128, :])
            for t in range(4):
                nc.sync.dma_start(w2h[t][:], w2_d[t * 64:(t + 1) * 64, :])

            # ---- persistent intermediates ----
            qkTs = [bigpool.tile([128, SEQ], MMDT, tag=f"qkT{m}",
                                 name=f"qkT{m}") for m in range(4)]
            # per head: 64 V columns + 64 replicated ones columns -> the AV
            # matmul emits the softmax denominator replicated on output
            # partitions 64:128 (lane-aligned for reciprocal + DMA lane-move)
            vaugs = [bigpool.tile([128, HPC * 128], MMDT, tag=f"vaug{st}",
                                  name=f"vaug{st}") for st in range(NKC)]
            # oTA: head-half A (d on partitions 0:64, q free, pair-packed)
            oTA = bigpool.tile([64, 2 * SEQ], MMDT, tag="oTA")
            oTB = bigpool.tile([64, 2 * SEQ], MMDT, tag="oTB")

            def vaug_t(kc):
                return vaugs[kc].rearrange("p (h e) -> p h e", e=128)

            # ---- phase 1: projections ----
            # proj / outproj psums borrow the "s" tag slots (same bank size);
            # lets scores run 3 buffers deep within the 8-bank PSUM budget
            def qk_unit(m, s2):
                pp = ps.tile([128, 2 * QCH], F32, tag="s", bufs=3,
                             name="pp")[:, 0:512]
                for c in range(NCC):
                    nc.tensor.matmul(
                        pp[:],
                        wps[c][:, m * 128:(m + 1) * 128],
                        xTs[c][:, s2 * 512:(s2 + 1) * 512],
                        start=(c == 0), stop=(c == NCC - 1),
                        skip_group_check=True,
                    )
                nc.any.tensor_copy(qkTs[m][:, s2 * 512:(s2 + 1) * 512], pp[:])

            def v_unit(st):
                pv = ps.tile([128, 2 * QCH], F32, tag="s", bufs=3,
                             name="pv")[:, 0:HPC * HD]
                for c in range(NCC):
                    nc.tensor.matmul(
                        pv[:],
                        xTs[c][:, st * 128:(st + 1) * 128],
                        wvs[c][:],
                        start=(c == 0), stop=(c == NCC - 1),
                        skip_group_check=True,
                    )
                vt = vaug_t(st)
                nc.vector.tensor_copy(
                    vt[:, :, 0:64], pv[:].rearrange("p (h d) -> p h d", d=HD))
                nc.vector.tensor_copy(
                    vt[:, :, 64:128],
                    ones4[:].rearrange("p (h o) -> p h o", o=64))

            for m in range(2):
                for s2 in range(NQC):
                    qk_unit(m, s2)
            for st in range(NKC):
                v_unit(st)

            filler = [lambda m=m, s2=s2: qk_unit(m, s2)
                      for m in (2, 3) for s2 in range(NQC)]

            # ---- out-projection (one q-tile of 128) ----
            def emit_outproj_qt(qt):
                yps = ps.tile([128, 2 * QCH], F32, tag="s", bufs=3,
                              name="yps")[:, 0:DIM]
                for t, oTx in enumerate((oTA, oTB, oTA, oTB)):
                    pp = t // 2  # pair
                    nc.tensor.matmul(
                        yps[:],
                        oTx[:, pp * SEQ + qt * 128:
                            pp * SEQ + (qt + 1) * 128],
                        w2h[t][:],
                        start=(t == 0), stop=(t == 3),
                        skip_group_check=True,
                    )
                ya = ypool.tile([128, DIM], F32, tag="ya")
                nc.any.tensor_copy(ya[:], yps[:])
                nc.sync.dma_start(y_d[qt * 128:(qt + 1) * 128, :], ya[:])

            # ---- phase 2: attention ----
            # deferred block-end work: emitted a few i's into the NEXT block
            # so recip/DMA/scale never head-of-line-block the DVE FIFO
            pending = []

            def emit_pending_recip(pend):
                nc.vector.reciprocal_approx_fast(
                    out=pend["rt"][:], in_=pend["den"][:])
                nc.sync.dma_start(pend["rtb"][:, 0:QCH],
                                  pend["rt"][64:128, 0:QCH])
                nc.sync.dma_start(pend["rtb"][:, QCH:2 * QCH],
                                  pend["rt"][64:128, QCH:2 * QCH])

            def emit_pending_scale(pend, half):
                pp_, qcc = pend["p"], pend["qc"]
                cols = slice(pp_ * SEQ + qcc * QCH,
                             pp_ * SEQ + (qcc + 1) * QCH)
                if half == 0:
                    nc.vector.tensor_mul(
                        oTA[:, cols], pend["den"][0:64, 0:QCH],
                        pend["rtb"][:, 0:QCH])
                else:
                    nc.vector.tensor_mul(
                        oTB[:, cols], pend["den"][0:64, QCH:2 * QCH],
                        pend["rtb"][:, QCH:2 * QCH])

            def drain_pending(n=99):
                cnt = 0
                while pending and cnt < n:
                    pend = pending[0]
                    st = pend["stage"]
                    if st == 0:
                        emit_pending_recip(pend)
                    elif st == 1:
                        emit_pending_scale(pend, 0)
                    else:
                        emit_pending_scale(pend, 1)
                        pending.pop(0)
                    pend["stage"] = st + 1
                    cnt += 1

            for p in range(2):
                Q = qkTs[2 * p]
                K = qkTs[2 * p + 1]
                for qc in range(NQC):
                    oA = ps.tile([128, QCH], F32, tag="oA", bufs=1, name="oA")
                    oB = ps.tile([128, QCH], F32, tag="oB", bufs=1, name="oB")

                    def emit_av(kc, pstr):
                        nc.tensor.matmul(
                            oA[:], vaug_t(kc)[:, 2 * p, :],
                            pstr[:, 0:QCH],
                            start=(kc == 0), stop=(kc == NKC - 1),
                            skip_group_check=True,
                        )
                        nc.tensor.matmul(
                            oB[:], vaug_t(kc)[:, 2 * p + 1, :],
                            pstr[:, QCH:2 * QCH],
                            start=(kc == 0), stop=(kc == NKC - 1),
                            skip_group_check=True,
                        )

                    def emit_scores(kc):
                        s = ps.tile([128, 2 * QCH], F32, tag="s", bufs=3,
                                    name="s")
                        nc.tensor.matmul(
                            s[:, 0:QCH],
                            K[0:64, kc * 128:(kc + 1) * 128],
                            Q[0:64, qc * QCH:(qc + 1) * QCH],
                            start=True, stop=True, skip_group_check=True,
                            tile_position=(0, 0),
                        )
                        nc.tensor.matmul(
                            s[:, QCH:2 * QCH],
                            K[64:128, kc * 128:(kc + 1) * 128],
                            Q[64:128, qc * QCH:(qc + 1) * QCH],
                            start=True, stop=True, skip_group_check=True,
                            tile_position=(64, 0),
                        )
                        return s

                    def emit_exp(i, s):
                        if _dve_slot(p, qc, i):
                            pt = ptpool.tile([128, 2 * QCH], I16, tag="p",
                                             name="pt")
                            nc.vector.tensor_scalar(
                                out=pt[:], in0=s[:],
                                scalar1=TRICK_A, scalar2=TRICK_B,
                                op0=mybir.AluOpType.mult,
                                op1=mybir.AluOpType.add,
                            )
                            return pt[:].bitcast(BF16)
                        pb = ptpool.tile([128, 2 * QCH], BF16, tag="p",
                                         name="pb")
                        nc.scalar.activation(
                            pb[:], s[:], mybir.ActivationFunctionType.Exp,
                            scale=SCALE)
                        return pb[:]

                    # 2-kc batches: [scores x4 in 64-row mode][AV x4 + other
                    # 128-mode work] halves the PE mode-switch drains
                    prev = [None, None]
                    for ii in range(NKC // 2):
                        i0, i1 = 2 * ii, 2 * ii + 1
                        s0 = emit_scores(i0)
                        s1 = emit_scores(i1)
                        if prev[0] is not None:
                            emit_av(i0 - 2, prev[0])
                            emit_av(i1 - 2, prev[1])
                        if filler and ii % 2 == 1:
                            filler.pop(0)()
                        if ii in (1, 2, 4):
                            drain_pending(1)
                        if p == 1 and qc > 0 and ii in (5, 6, 7):
                            j = (5, 6, 7).index(ii)
                            emit_outproj_qt((qc - 1) * (QCH // 128) + j)
                            if ii == 7:
                                emit_outproj_qt((qc - 1) * (QCH // 128) + 3)
                        prev = [emit_exp(i0, s0), emit_exp(i1, s1)]
                    emit_av(NKC - 2, prev[0])
                    emit_av(NKC - 1, prev[1])
                    # ---- block end: only the bank-freeing CASTs stay here.
                    # den rows 0:64 hold O, rows 64:128 the replicated
                    # denominator.  reciprocal (base-0 custom DVE op), DMA
                    # lane-move (partitions 64:128 -> 0:64) and the scale-muls
                    # run deferred inside the NEXT block via drain_pending(),
                    # so they never head-of-line-block the DVE FIFO.
                    den = smallpool.tile([128, 2 * QCH], F32, tag="den")
                    nc.vector.tensor_copy(den[:, 0:QCH], oA[:])
                    nc.vector.tensor_copy(den[:, QCH:2 * QCH], oB[:])
                    pending.append({
                        "p": p, "qc": qc, "stage": 0, "den": den,
                        "rt": smallpool.tile([128, 2 * QCH], F32, tag="rt", name="rt"),
                        "rtb": smallpool.tile([64, 2 * QCH], F32, tag="rtb", name="rtb"),
                    })

            # tail: last block's evac chain + out-projection
            drain_pending()
            for qt in range((NQC - 1) * (QCH // 128), NQC * (QCH // 128)):
                emit_outproj_qt(qt)

    nc.finalize()
    return nc


_NC_CACHE = {}


def get_nc():
    if "nc" not in _NC_CACHE:
        _NC_CACHE["nc"] = build_nc()
    return _NC_CACHE["nc"]


def make_core_inputs(x, w_qkv, w_out):
    """Per-core input dicts (host-side sharding)."""
    import ml_dtypes
    mmnp = ml_dtypes.bfloat16
    in_maps = []
    for c in range(NCORES):
        b, hg = c // 2, c % 2
        heads = [hg * HPC + i for i in range(HPC)]
        qcols = [w_qkv[:, h * HD:(h + 1) * HD] for h in heads]
        kcols = [w_qkv[:, DIM + h * HD:DIM + (h + 1) * HD] for h in heads]
        vcols = [w_qkv[:, 2 * DIM + h * HD:2 * DIM + (h + 1) * HD]
                 for h in heads]
        wperm = np.concatenate(
            [qcols[0], qcols[1], kcols[0], kcols[1],
             qcols[2], qcols[3], kcols[2], kcols[3]], axis=1)
        wv = np.concatenate(vcols, axis=1)
        w2 = w_out[hg * HPC * HD:(hg + 1) * HPC * HD, :]
        in_maps.append({
            "xt": np.ascontiguousarray(x[b].T).astype(mmnp),
            "wperm": np.ascontiguousarray(wperm).astype(mmnp),
            "wv": np.ascontiguousarray(wv).astype(mmnp),
            "w2": np.ascontiguousarray(w2).astype(mmnp),
        })
    return in_maps


def kernel(x, w_qkv, w_out, b_out):
    from concourse.bass_utils import run_bass_kernel_spmd

    x = np.asarray(x, dtype=np.float32)
    w_qkv = np.asarray(w_qkv, dtype=np.float32)
    w_out = np.asarray(w_out, dtype=np.float32)
    b_out = np.asarray(b_out, dtype=np.float32)

    nc = get_nc()
    in_maps = make_core_inputs(x, w_qkv, w_out)
    res = run_bass_kernel_spmd(nc, in_maps, list(range(NCORES))).results

    out = np.empty((B, SEQ, DIM), dtype=np.float32)
    for b in range(B):
        out[b] = res[2 * b]["y"] + res[2 * b + 1]["y"] + b_out
    return out


# revision 37
# speedup vs baseline: 1.0404x; 1.0404x over previous
"""Multi-head self-attention forward on 8 Trainium2 NeuronCores.

Problem: x[4,2048,512] -> qkv proj (w_qkv [512,1536]) -> 8-head attention
(head_dim 64) -> out proj (w_out [512,512] + b_out) -> y[4,2048,512].

Sharding: 8 shards = (batch b in 0..3) x (head-group hg in 0..1, 4 heads each).
Core c handles b=c//2, hg=c%2. Host sums the two half-projections per batch
and adds the bias.

Kernel structure (per core):
  phase 1: qkT = wperm.T @ xT (4 tiles [128,2048]: Q01,K01,Q23,K23 with the
    pair's two heads stacked on partitions 0:64 / 64:128), v_aug seq-tiles
    [128, 4*65] (per head 64 v columns + ones column -> the AV matmul also
    emits the softmax denominator as output row 64).
  phase 2: per (pair p, q-chunk qc of 512): 16 kc iterations:
    - scores: two row-tiled (64x128 PE mode) matmuls, head A -> s[:,0:512],
      head B -> s[:,512:1024]; the 64-row tiles at tile_position (0,0)/(64,0)
      execute concurrently.
    - exp: split between ACT (activation Exp) and DVE (Schraudolph bit-trick:
      one tensor_scalar mul+add emitting int16 bf16-bit-patterns, bitcast to
      bf16) so neither engine is the bottleneck.
    - AV: 2 matmuls accumulate [V|1].T @ p into oA/oB [65, 512] psum.
    block end: rt = 1/den via DVE reciprocal from psum row 64, DMA broadcasts
    rt across 64 partitions, one fused DVE tensor_tensor per head scales the
    oA/oB psum by rt while evacuating to bf16 oT tiles (partitions 0:64).
  out-proj: per q-tile of 128: 4 accumulating K=64 matmuls (oT blocks x w2
    head-half rows), evac + DMA. Emitted after each pair-1 block (and tail).
"""

import math

import numpy as np

import concourse.bass as bass
import concourse.mybir as mybir
import concourse.tile as tile
from concourse import bacc

DIM = 512
NHEADS = 8
HD = 64
B = 4
SEQ = 2048
SCALE = HD ** -0.5

NCORES = 8
HPC = 4          # heads per core
QCH = 512        # q chunk
NQC = SEQ // QCH # 4 q-chunks
KCH = 128        # k chunk (psum partition dim)
NKC = SEQ // KCH # 16 k-chunks
CCH = 128        # contraction chunk for projections
NCC = DIM // CCH # 4

F32 = mybir.dt.float32
BF16 = mybir.dt.bfloat16
I16 = mybir.dt.int16
MMDT = BF16

# Schraudolph exp in bf16 bit-space: exp(s*SCALE) ~= bitcast_bf16(int16(
#   s * TRICK_A + TRICK_B)).  TRICK_A = 128*log2(e)*SCALE.  C_CORR centers
# the sawtooth approximation error (tuned vs rel-err on HW).
C_CORR = 7.33
TRICK_A = float(128.0 / math.log(2.0) * SCALE)
TRICK_B = float(127.0 * 128.0 - C_CORR)


def _dve_slot(p, qc, i):
    """Which exp units run on DVE (Schraudolph) vs ACT (exact).  The last
    two i keep DVE free for the block-end reciprocal/scale chain."""
    if i >= NKC - 2:
        return False
    return i % 8 in (1, 4, 6) if (p, qc) != (0, 0) else i in (5, 9, 13)


def build_nc():
    nc = bacc.Bacc()

    xT_d = nc.dram_tensor("xt", [DIM, SEQ], MMDT, kind="ExternalInput")
    wperm_d = nc.dram_tensor("wperm", [DIM, 4 * 128], MMDT, kind="ExternalInput")
    wv_d = nc.dram_tensor("wv", [DIM, HPC * HD], MMDT, kind="ExternalInput")
    w2_d = nc.dram_tensor("w2", [HPC * HD, DIM], MMDT, kind="ExternalInput")
    y_d = nc.dram_tensor("y", [SEQ, DIM], F32, kind="ExternalOutput")

    with tile.TileContext(nc) as tc:
        with (
            tc.tile_pool(name="const", bufs=1) as cpool,
            tc.tile_pool(name="big", bufs=1) as bigpool,
            tc.tile_pool(name="pt", bufs=4) as ptpool,
            tc.tile_pool(name="small", bufs=2) as smallpool,
            tc.tile_pool(name="ysb", bufs=2) as ypool,
            tc.tile_pool(name="ps", bufs=1, space="PSUM") as ps,
        ):
            # ---- constants / inputs to SBUF ----
            xTs = [cpool.tile([128, SEQ], MMDT, tag=f"xT{c}", name=f"xT{c}")
                   for c in range(NCC)]
            wps = [cpool.tile([128, 512], MMDT, tag=f"wp{c}", name=f"wp{c}")
                   for c in range(NCC)]
            wvs = [cpool.tile([128, HPC * HD], MMDT, tag=f"wv{c}", name=f"wv{c}")
                   for c in range(NCC)]
            # w2 head-half rows, each on partitions 0:64
            w2h = [cpool.tile([64, DIM], MMDT, tag=f"w2h{t}", name=f"w2h{t}")
                   for t in range(4)]
            ones4 = cpool.tile([128, HPC * 64], F32, tag="ones4")
            nc.gpsimd.memset(ones4[:], 1.0)
            ones1 = cpool.tile([1, 1], F32, tag="ones1")
            nc.gpsimd.memset(ones1[:], 1.0)
            # preload exp ACT tables so the first attention exp doesn't stall
            dummy = cpool.tile([1, 1], F32, tag="dummy")
            nc.scalar.activation(dummy[:], ones1[:],
                                 mybir.ActivationFunctionType.Exp)

            nc.gpsimd.dma_start(wps[0][:, 0:128], wperm_d[0:128, 0:128])
            nc.gpsimd.dma_start(wps[0][:, 128:512], wperm_d[0:128, 128:512])
            nc.gpsimd.dma_start(xTs[0][:, 0:512], xT_d[0:128, 0:512])
            for c in range(NCC):
                nc.sync.dma_start(xTs[c][:, 0 if c else 512:1024],
                                  xT_d[c * 128:(c + 1) * 128, 0 if c else 512:1024])
            for c in range(1, NCC):
                nc.sync.dma_start(wps[c][:], wperm_d[c * 128:(c + 1) * 128, :])
            for c in range(NCC):
                nc.sync.dma_start(xTs[c][:, 1024:SEQ],
                                  xT_d[c * 128:(c + 1) * 128, 1024:SEQ])
                nc.sync.dma_start(wvs[c][:], wv_d[c * 128:(c + 1) * 128, :])
            for t in range(4):
                nc.sync.dma_start(w2h[t][:], w2_d[t * 64:(t + 1) * 64, :])

            # ---- persistent intermediates ----
            qkTs = [bigpool.tile([128, SEQ], MMDT, tag=f"qkT{m}",
                                 name=f"qkT{m}") for m in range(4)]
            # per head: 64 V columns + 64 replicated ones columns -> the AV
            # matmul emits the softmax denominator replicated on output
            # partitions 64:128 (lane-aligned for reciprocal + DMA lane-move)
            vaugs = [bigpool.tile([128, HPC * 128], MMDT, tag=f"vaug{st}",
                                  name=f"vaug{st}") for st in range(NKC)]
            # oTA: head-half A (d on partitions 0:64, q free, pair-packed)
            oTA = bigpool.tile([64, 2 * SEQ], MMDT, tag="oTA")
            oTB = bigpool.tile([64, 2 * SEQ], MMDT, tag="oTB")

            def vaug_t(kc):
                return vaugs[kc].rearrange("p (h e) -> p h e", e=128)

            # ---- phase 1: projections ----
            # proj / outproj psums borrow the "s" tag slots (same bank size);
            # lets scores run 3 buffers deep within the 8-bank PSUM budget
            def qk_unit(m, s2):
                pp = ps.tile([128, 2 * QCH], F32, tag="s", bufs=3,
                             name="pp")[:, 0:512]
                for c in range(NCC):
                    nc.tensor.matmul(
                        pp[:],
                        wps[c][:, m * 128:(m + 1) * 128],
                        xTs[c][:, s2 * 512:(s2 + 1) * 512],
                        start=(c == 0), stop=(c == NCC - 1),
                        skip_group_check=True,
                    )
                nc.any.tensor_copy(qkTs[m][:, s2 * 512:(s2 + 1) * 512], pp[:])

            def v_unit(st):
                pv = ps.tile([128, 2 * QCH], F32, tag="s", bufs=3,
                             name="pv")[:, 0:HPC * HD]
                for c in range(NCC):
                    nc.tensor.matmul(
                        pv[:],
                        xTs[c][:, st * 128:(st + 1) * 128],
                        wvs[c][:],
                        start=(c == 0), stop=(c == NCC - 1),
                        skip_group_check=True,
                    )
                vt = vaug_t(st)
                nc.vector.tensor_copy(
                    vt[:, :, 0:64], pv[:].rearrange("p (h d) -> p h d", d=HD))
                nc.vector.tensor_copy(
                    vt[:, :, 64:128],
                    ones4[:].rearrange("p (h o) -> p h o", o=64))

            for m in range(2):
                for s2 in range(NQC):
                    qk_unit(m, s2)
            for st in range(NKC):
                v_unit(st)

            filler = [lambda m=m, s2=s2: qk_unit(m, s2)
                      for m in (2, 3) for s2 in range(NQC)]

            # ---- out-projection (one q-tile of 128) ----
            def emit_outproj_qt(qt):
                yps = ps.tile([128, 2 * QCH], F32, tag="s", bufs=3,
                              name="yps")[:, 0:DIM]
                for t, oTx in enumerate((oTA, oTB, oTA, oTB)):
                    pp = t // 2  # pair
                    nc.tensor.matmul(
                        yps[:],
                        oTx[:, pp * SEQ + qt * 128:
                            pp * SEQ + (qt + 1) * 128],
                        w2h[t][:],
                        start=(t == 0), stop=(t == 3),
                        skip_group_check=True,
                    )
                ya = ypool.tile([128, DIM], F32, tag="ya")
                nc.any.tensor_copy(ya[:], yps[:])
                nc.sync.dma_start(y_d[qt * 128:(qt + 1) * 128, :], ya[:])

            # ---- phase 2: attention ----
            # deferred block-end work: emitted a few i's into the NEXT block
            # so recip/DMA/scale never head-of-line-block the DVE FIFO
            pending = []

            def emit_pending_recip(pend):
                nc.vector.reciprocal_approx_fast(
                    out=pend["rt"][:], in_=pend["den"][:])
                nc.sync.dma_start(pend["rtb"][:, 0:QCH],
                                  pend["rt"][64:128, 0:QCH])
                nc.sync.dma_start(pend["rtb"][:, QCH:2 * QCH],
                                  pend["rt"][64:128, QCH:2 * QCH])

            def emit_pending_scale(pend, half):
                pp_, qcc = pend["p"], pend["qc"]
                cols = slice(pp_ * SEQ + qcc * QCH,
                             pp_ * SEQ + (qcc + 1) * QCH)
                if half == 0:
                    nc.vector.tensor_mul(
                        oTA[:, cols], pend["den"][0:64, 0:QCH],
                        pend["rtb"][:, 0:QCH])
                else:
                    nc.vector.tensor_mul(
                        oTB[:, cols], pend["den"][0:64, QCH:2 * QCH],
                        pend["rtb"][:, QCH:2 * QCH])

            def drain_pending(n=99):
                cnt = 0
                while pending and cnt < n:
                    pend = pending[0]
                    st = pend["stage"]
                    if st == 0:
                        emit_pending_recip(pend)
                    elif st == 1:
                        emit_pending_scale(pend, 0)
                    else:
                        emit_pending_scale(pend, 1)
                        pending.pop(0)
                    pend["stage"] = st + 1
                    cnt += 1

            for p in range(2):
                Q = qkTs[2 * p]
                K = qkTs[2 * p + 1]
                for qc in range(NQC):
                    oA = ps.tile([128, QCH], F32, tag="oA", bufs=1, name="oA")
                    oB = ps.tile([128, QCH], F32, tag="oB", bufs=1, name="oB")

                    def emit_av(kc, pstr):
                        nc.tensor.matmul(
                            oA[:], vaug_t(kc)[:, 2 * p, :],
                            pstr[:, 0:QCH],
                            start=(kc == 0), stop=(kc == NKC - 1),
                            skip_group_check=True,
                        )
                        nc.tensor.matmul(
                            oB[:], vaug_t(kc)[:, 2 * p + 1, :],
                            pstr[:, QCH:2 * QCH],
                            start=(kc == 0), stop=(kc == NKC - 1),
                            skip_group_check=True,
                        )

                    def emit_scores(kc):
                        s = ps.tile([128, 2 * QCH], F32, tag="s", bufs=3,
                                    name="s")
                        nc.tensor.matmul(
                            s[:, 0:QCH],
                            K[0:64, kc * 128:(kc + 1) * 128],
                            Q[0:64, qc * QCH:(qc + 1) * QCH],
                            start=True, stop=True, skip_group_check=True,
                            tile_position=(0, 0),
                        )
                        nc.tensor.matmul(
                            s[:, QCH:2 * QCH],
                            K[64:128, kc * 128:(kc + 1) * 128],
                            Q[64:128, qc * QCH:(qc + 1) * QCH],
                            start=True, stop=True, skip_group_check=True,
                            tile_position=(64, 0),
                        )
                        return s

                    def emit_exp(i, s):
                        if _dve_slot(p, qc, i):
                            pt = ptpool.tile([128, 2 * QCH], I16, tag="p",
                                             name="pt")
                            nc.vector.tensor_scalar(
                                out=pt[:], in0=s[:],
                                scalar1=TRICK_A, scalar2=TRICK_B,
                                op0=mybir.AluOpType.mult,
                                op1=mybir.AluOpType.add,
                            )
                            return pt[:].bitcast(BF16)
                        pb = ptpool.tile([128, 2 * QCH], BF16, tag="p",
                                         name="pb")
                        nc.scalar.activation(
                            pb[:], s[:], mybir.ActivationFunctionType.Exp,
                            scale=SCALE)
                        return pb[:]

                    # 2-kc batches: [scores x4 in 64-row mode][AV x4 + other
                    # 128-mode work] halves the PE mode-switch drains
                    prev = [None, None]
                    for ii in range(NKC // 2):
                        i0, i1 = 2 * ii, 2 * ii + 1
                        s0 = emit_scores(i0)
                        s1 = emit_scores(i1)
                        if prev[0] is not None:
                            emit_av(i0 - 2, prev[0])
                            emit_av(i1 - 2, prev[1])
                        if filler and ii % 2 == 1:
                            filler.pop(0)()
                        if ii in (1, 2, 4):
                            drain_pending(1)
                        if p == 1 and qc > 0 and ii in (5, 6, 7):
                            j = (5, 6, 7).index(ii)
                            emit_outproj_qt((qc - 1) * (QCH // 128) + j)
                            if ii == 7:
                                emit_outproj_qt((qc - 1) * (QCH // 128) + 3)
                        prev = [emit_exp(i0, s0), emit_exp(i1, s1)]
                    emit_av(NKC - 2, prev[0])
                    emit_av(NKC - 1, prev[1])
                    # ---- block end: only the bank-freeing CASTs stay here.
                    # den rows 0:64 hold O, rows 64:128 the replicated
                    # denominator.  reciprocal (base-0 custom DVE op), DMA
                    # lane-move (partitions 64:128 -> 0:64) and the scale-muls
                    # run deferred inside the NEXT block via drain_pending(),
                    # so they never head-of-line-block the DVE FIFO.
                    den = smallpool.tile([128, 2 * QCH], F32, tag="den")
                    nc.vector.tensor_copy(den[:, 0:QCH], oA[:])
                    nc.vector.tensor_copy(den[:, QCH:2 * QCH], oB[:])
                    pending.append({
                        "p": p, "qc": qc, "stage": 0, "den": den,
                        "rt": smallpool.tile([128, 2 * QCH], F32, tag="rt", name="rt"),
                        "rtb": smallpool.tile([64, 2 * QCH], F32, tag="rtb", name="rtb"),
                    })

            # tail: last block's evac chain + out-projection
            drain_pending()
            for qt in range((NQC - 1) * (QCH // 128), NQC * (QCH // 128)):
                emit_outproj_qt(qt)

    nc.finalize()
    return nc


_NC_CACHE = {}


def get_nc():
    if "nc" not in _NC_CACHE:
        _NC_CACHE["nc"] = build_nc()
    return _NC_CACHE["nc"]


def make_core_inputs(x, w_qkv, w_out):
    """Per-core input dicts (host-side sharding)."""
    import ml_dtypes
    mmnp = ml_dtypes.bfloat16
    in_maps = []
    for c in range(NCORES):
        b, hg = c // 2, c % 2
        heads = [hg * HPC + i for i in range(HPC)]
        qcols = [w_qkv[:, h * HD:(h + 1) * HD] for h in heads]
        kcols = [w_qkv[:, DIM + h * HD:DIM + (h + 1) * HD] for h in heads]
        vcols = [w_qkv[:, 2 * DIM + h * HD:2 * DIM + (h + 1) * HD]
                 for h in heads]
        wperm = np.concatenate(
            [qcols[0], qcols[1], kcols[0], kcols[1],
             qcols[2], qcols[3], kcols[2], kcols[3]], axis=1)
        wv = np.concatenate(vcols, axis=1)
        w2 = w_out[hg * HPC * HD:(hg + 1) * HPC * HD, :]
        in_maps.append({
            "xt": np.ascontiguousarray(x[b].T).astype(mmnp),
            "wperm": np.ascontiguousarray(wperm).astype(mmnp),
            "wv": np.ascontiguousarray(wv).astype(mmnp),
            "w2": np.ascontiguousarray(w2).astype(mmnp),
        })
    return in_maps


def kernel(x, w_qkv, w_out, b_out):
    from concourse.bass_utils import run_bass_kernel_spmd

    x = np.asarray(x, dtype=np.float32)
    w_qkv = np.asarray(w_qkv, dtype=np.float32)
    w_out = np.asarray(w_out, dtype=np.float32)
    b_out = np.asarray(b_out, dtype=np.float32)

    nc = get_nc()
    in_maps = make_core_inputs(x, w_qkv, w_out)
    res = run_bass_kernel_spmd(nc, in_maps, list(range(NCORES))).results

    out = np.empty((B, SEQ, DIM), dtype=np.float32)
    for b in range(B):
        out[b] = res[2 * b]["y"] + res[2 * b + 1]["y"] + b_out
    return out
